# revision 19
# baseline (speedup 1.0000x reference)
"""Trainium2 Bass kernel for nn_CSPVNet (GNN message passing), 8 NeuronCores.

Strategy:
  - Sort edges by src on host; each core owns a contiguous node range and all
    edges whose src falls in it (scatter-mean is then core-local).
  - Per layer only the dst-side projection table B = hn @ W1b.T is exchanged
    (AllGather, fp16). hi/hj enter the edge MLP additively, so per-node
    projections A[src], B[dst] replace the per-edge concat matmul; the vij
    term folds into a [3,128] matrix on dv.
  - A[src] is expanded on-chip with interval one-hot matmuls (src sorted ->
    each node's edges are a contiguous run); B[dst] is fetched with indirect
    DMA gathers and transpose-accumulated into the PSUM accumulator.
  - Scatter-mean: per 128-edge chunk, equality one-hot matmul into PSUM,
    scaled by host-computed 1/deg.
"""
import math
import os
import time

import numpy as np

N = 50000
E = 800000
G = 1000
H = 128
NF = 10
DIS = 60
L = 4
TD = 128
EPS = 1e-5

NCORES = 8
P = 128
NPCR = N // NCORES          # 6250 real nodes per core
NPC = 6272                  # padded nodes per core (49*128)
NODE_TILES = NPC // 512     # 12.25 -> not integer! 6272/512 = 12.25
GPAD = 1152                 # padded graph count (9*128)

assert NPC % P == 0
# node tiles of 512 except we keep 6272 = 12*512 + 128 -> use 512-tiles plus
# one 128 tile; simpler: use node tiles of 448? Use tile list instead.
NT_NODE = [(i * 512, 512) for i in range(12)] + [(6144, 128)]


# ---------------------------------------------------------------------------
# host preprocessing
# ---------------------------------------------------------------------------

def _f16(x):
    return np.ascontiguousarray(x, dtype=np.float16)


def _f32(x):
    return np.ascontiguousarray(x, dtype=np.float32)


def preprocess(inputs):
    t = _f32(inputs["t"]).reshape(-1)          # [G]
    pos = _f32(inputs["pos"])                  # [N,3]
    v = _f32(inputs["v"])                      # [N,3]
    l = _f32(inputs["l"])                      # [G,6]
    h = np.asarray(inputs["h"]).astype(np.int64).reshape(-1)
    node_index = np.asarray(inputs["node_index"]).astype(np.int64).reshape(-1)
    eni = np.asarray(inputs["edge_node_index"]).astype(np.int64)
    src_all, dst_all = eni[0], eni[1]

    order = np.argsort(src_all, kind="stable")
    src_s = src_all[order].astype(np.int64)
    dst_s = dst_all[order].astype(np.int64)

    deg = np.bincount(src_all, minlength=N)

    # window structure
    WN = 116                                   # nodes per window
    NWIN = math.ceil(NPC / WN)                 # windows per core
    # window w of core c covers local nodes [w*WN, min((w+1)*WN, NPC))
    # count edges per (core, window)
    maxcnt = 0
    for c in range(NCORES):
        lo, hi = c * NPCR, (c + 1) * NPCR
        # local node of each node
        for w in range(NWIN):
            a = c * NPCR + w * WN
            b = min(c * NPCR + (w + 1) * WN, (c + 1) * NPCR)
            if a >= b:
                cnt = 0
            else:
                cnt = int(deg[a:b].sum())
            maxcnt = max(maxcnt, cnt)
    TPC = max(1, math.ceil(maxcnt / 512))      # tiles per window
    NT = NWIN * TPC                            # edge tiles per core
    EPC = NT * 512

    # global padded B-row index of node n
    def gp(n):
        return (n // NPCR) * NPC + (n % NPCR)

    gp_dst = (dst_s // NPCR) * NPC + (dst_s % NPCR)

    # per-core arrays
    meta = np.zeros((NCORES, NT, 128, 8), np.float32)
    meta[..., 0:4] = -1.0
    bidx = np.zeros((NCORES, NT, 128, 4), np.int32)
    stat_pos = np.zeros((NCORES, NT, 2, 3, 512), np.float32)
    stat_ldv = np.zeros((NCORES, NT, 2, 12, 512), np.float32)
    recipdeg = np.zeros((NCORES, 128, NWIN), np.float32)
    gidxw = np.zeros((NCORES, 49, 128, 3), np.float32) - 1
    # gidxw is per node-chunk (128-aligned chunks for the graph phase): 49
    gbase = np.zeros((NCORES, 8), np.int32)
    tidx = np.zeros((NCORES, 49, 128, 1), np.int32)
    embfm = np.zeros((NCORES, 128, NPC), np.float16)
    recipg = np.zeros((GPAD, 1), np.float32)
    trow = np.zeros((1, 1024), np.float32)
    trow[0, :G] = t

    gcount = np.bincount(node_index, minlength=G)
    recipg[:G, 0] = 1.0 / np.maximum(gcount, 1.0)

    # edge starts per node (within sorted edge list)
    nstart = np.zeros(N + 1, np.int64)
    np.cumsum(deg, out=nstart[1:])

    l_edge_all = l[node_index[src_s]]          # [E,6] host gather
    pd_dst = pos[dst_s]
    pd_src = pos[src_s]
    v_dst = v[dst_s]
    v_src = v[src_s]

    for c in range(NCORES):
        n0 = c * NPCR
        # per-window packing
        for w in range(NWIN):
            a = n0 + w * WN
            b = min(n0 + (w + 1) * WN, n0 + NPCR)
            if a < b:
                e0, e1 = int(nstart[a]), int(nstart[b])
            else:
                e0 = e1 = 0
            cnt = e1 - e0
            # tile-local placement
            tbase = w * TPC
            for ti in range(TPC):
                tt = tbase + ti
                s = e0 + ti * 512
                m = min(512, e1 - s) if s < e1 else 0
                if m <= 0:
                    continue
                sl = slice(s, s + m)
                eidx = np.arange(m)
                prt = eidx % 128
                col = eidx // 128
                # src local in window
                meta[c, tt, prt, col] = (src_s[sl] - a).astype(np.float32)
                bidx[c, tt, prt, col] = gp_dst[sl]
                # feature-major statics
                stat_pos[c, tt, 0, :, :m] = pd_dst[sl].T
                stat_pos[c, tt, 1, :, :m] = pd_src[sl].T
                stat_ldv[c, tt, 0, 0:6, :m] = l_edge_all[sl].T
                stat_ldv[c, tt, 0, 6:9, :m] = v_dst[sl].T
                stat_ldv[c, tt, 1, 6:9, :m] = v_src[sl].T
            # interval ranges for the A-expand onehot: rows are window rows
            # (0..127); row r covers edges [nstart[a+r]-e0, nstart[a+r+1]-e0)
            # clipped per tile.
            for ti in range(TPC):
                tt = tbase + ti
                t0 = ti * 512
                nn = b - a
                if nn <= 0:
                    continue
                st = nstart[a:a + nn] - e0 - t0
                en = nstart[a + 1:a + nn + 1] - e0 - t0
                meta[c, tt, :nn, 4] = np.clip(st, 0, 512).astype(np.float32)
                meta[c, tt, :nn, 5] = np.clip(en, 0, 512).astype(np.float32)
        # recip deg per window row
        for w in range(NWIN):
            a = n0 + w * WN
            b = min(n0 + (w + 1) * WN, n0 + NPCR)
            nn = max(0, b - a)
            if nn > 0:
                recipdeg[c, :nn, w] = (
                    1.0 / np.maximum(deg[a:b], 1.0)).astype(np.float32)
        # graph phase per 128-node chunk
        gidx_core = np.zeros(NPC, np.int64)
        gidx_core[:NPCR] = node_index[n0:n0 + NPCR]
        g_first = int(node_index[n0])
        gw_base = (g_first // 128) * 128
        gbase[c, 0] = gw_base
        for k in range(49):
            gg = gidx_core[k * 128:(k + 1) * 128]
            for w in range(3):
                rel = gg - (gw_base + w * 128)
                ok = (rel >= 0) & (rel < 128) & (np.arange(128) < (
                    NPCR - k * 128 if k * 128 < NPCR else 0))
                gidxw[c, k, :, w] = np.where(ok, rel, -1).astype(np.float32)
            tidx[c, k, :, 0] = gidx_core[k * 128:(k + 1) * 128]
        emb_n = inputs["node_emb_table"][h[n0:n0 + NPCR]]     # [6250,128]
        embfm[c, :, :NPCR] = _f16(np.asarray(emb_n).T)

    # sanity: gw windows must cover all graphs of the core
    for c in range(NCORES):
        n0 = c * NPCR
        g_last = int(node_index[n0 + NPCR - 1])
        assert g_last < gbase[c, 0] + 384, "graph window overflow"

    # ---------------- weight prepacking ----------------
    W = {k: _f32(inputs[k]) for k in
         ["time_B", "atom_w", "atom_b", "vproj_w", "vproj_b", "edge_w1",
          "edge_b1", "edge_w2", "edge_b2", "node_w1", "node_b1", "node_w2",
          "node_b2", "ln_g", "ln_b", "fln_g", "fln_b", "outv_w1", "outv_b1",
          "outv_w2", "outl_w"]}

    # edge_w1 columns: [hi 0:128, hj 128:256, l 256:262, vij 262:322, pd 322:382]
    # pd col (comp c, tpart): c*20 + tp ; tp<10 sin f=tp ; tp>=10 cos f=tp-10
    Wcomb = np.zeros((L, 80, 128), np.float16)
    b1fold = np.zeros((L, 128), np.float32)
    for i in range(L):
        e1w = W["edge_w1"][i]                 # [128, 382]
        W_pd = e1w[:, 322:382]                # [128, 60]
        W_l = e1w[:, 256:262]
        W_vij = e1w[:, 262:322]               # [128, 60]
        Wdv = W_vij @ W["vproj_w"][i]         # [128,60]@[60,3] = [128,3]
        b1fold[i] = W["edge_b1"][i] + W_vij @ W["vproj_b"][i]
        for comp in range(3):
            for f in range(NF):
                Wcomb[i, comp * 10 + f, :] = W_pd[:, comp * 20 + f]
                Wcomb[i, 32 + comp * 10 + f, :] = W_pd[:, comp * 20 + 10 + f]
        Wcomb[i, 64:70, :] = W_l.T
        Wcomb[i, 70:73, :] = Wdv.T
    W1ab = np.zeros((L, 128, 256), np.float16)
    for i in range(L):
        W1ab[i, :, 0:128] = W["edge_w1"][i][:, 0:128].T
        W1ab[i, :, 128:256] = W["edge_w1"][i][:, 128:256].T
    W2T = _f16(np.transpose(W["edge_w2"], (0, 2, 1)))          # [L,128,128]
    Wn1 = np.zeros((L, 2, 128, 128), np.float16)
    for i in range(L):
        Wn1[i, 0] = W["node_w1"][i][:, 0:128].T
        Wn1[i, 1] = W["node_w1"][i][:, 128:256].T
    Wn2 = _f16(np.transpose(W["node_w2"], (0, 2, 1)))
    atomw = np.zeros((2, 128, 128), np.float16)
    atomw[0] = W["atom_w"][:, 0:128].T
    atomw[1] = W["atom_w"][:, 128:256].T
    W1o = _f16(W["outv_w1"].T)                                 # [128,128]
    W2o = _f16(W["outv_w2"].T)                                 # [128,3]
    Wlo = _f16(W["outl_w"].T)                                  # [128,6]

    # biases packed as columns [128, nb]
    bias_cols = []

    def addb(x):
        col = np.zeros(128, np.float32)
        col[:len(x)] = x
        bias_cols.append(col)
        return len(bias_cols) - 1

    IB = {}
    for i in range(L):
        IB[f"b1_{i}"] = addb(b1fold[i])
        IB[f"b2_{i}"] = addb(W["edge_b2"][i])
        IB[f"bn1_{i}"] = addb(W["node_b1"][i])
        IB[f"bn2_{i}"] = addb(W["node_b2"][i])
        IB[f"lng_{i}"] = addb(W["ln_g"][i])
        IB[f"lnb_{i}"] = addb(W["ln_b"][i])
    IB["atomb"] = addb(W["atom_b"])
    IB["flng"] = addb(W["fln_g"])
    IB["flnb"] = addb(W["fln_b"])
    IB["b1o"] = addb(W["outv_b1"])
    biases = np.stack(bias_cols, axis=1).astype(np.float32)    # [128, NB]

    M30 = np.zeros((3, 30), np.float32)
    for comp in range(3):
        for f in range(NF):
            M30[comp, comp * 10 + f] = (2.0 ** f) * np.pi

    host = dict(
        WN=WN, NWIN=NWIN, TPC=TPC, NT=NT, IB=IB,
        meta=meta, bidx=bidx, stat_pos=stat_pos, stat_ldv=stat_ldv,
        recipdeg=recipdeg, gidxw=gidxw, gbase=gbase, tidx=tidx,
        embfm=embfm, recipg=recipg, trow=trow,
        Wcomb=Wcomb, W1ab=W1ab, W2T=W2T, Wn1=Wn1, Wn2=Wn2, atomw=atomw,
        W1o=W1o, W2o=W2o, Wlo=Wlo, biases=biases, M30=M30,
        timeB=_f32(W["time_B"]),
    )
    return host


# ---------------------------------------------------------------------------
# bass program
# ---------------------------------------------------------------------------

def build_program(host, layers=L, debug_outs=()):
    import concourse.bass as bass
    import concourse.mybir as mybir
    import concourse.tile as tile
    from concourse import bacc
    from concourse.masks import make_identity

    dt = mybir.dt
    NWIN, TPC, NT, WN = host["NWIN"], host["TPC"], host["NT"], host["WN"]
    IB = host["IB"]
    NB = host["biases"].shape[1]

    nc = bacc.Bacc("TRN2", target_bir_lowering=False, debug=False,
                   num_devices=NCORES)

    def dram_in(name, shape, dtype):
        return nc.dram_tensor(name, list(shape), dtype, kind="ExternalInput")

    meta_d = dram_in("meta", [NT, 128, 8], dt.float32)
    bidx_d = dram_in("bidx", [NT, 128, 4], dt.int32)
    spos_d = dram_in("spos", [NT, 2, 3, 512], dt.float32)
    sldv_d = dram_in("sldv", [NT, 2, 12, 512], dt.float32)
    rdeg_d = dram_in("rdeg", [128, NWIN], dt.float32)
    gidxw_d = dram_in("gidxw", [49, 128, 3], dt.float32)
    gbase_d = dram_in("gbase", [1, 8], dt.int32)
    tidx_d = dram_in("tidx", [49, 128, 1], dt.int32)
    embfm_d = dram_in("embfm", [128, NPC], dt.float16)
    recipg_d = dram_in("recipg", [GPAD, 1], dt.float32)
    trow_d = dram_in("trow", [1, 1024], dt.float32)
    wcomb_d = dram_in("wcomb", [L, 80, 128], dt.float16)
    w1ab_d = dram_in("w1ab", [L, 128, 256], dt.float16)
    w2t_d = dram_in("w2t", [L, 128, 128], dt.float16)
    wn1_d = dram_in("wn1", [L, 2, 128, 128], dt.float16)
    wn2_d = dram_in("wn2", [L, 128, 128], dt.float16)
    atomw_d = dram_in("atomw", [2, 128, 128], dt.float16)
    w1o_d = dram_in("w1o", [128, 128], dt.float16)
    w2o_d = dram_in("w2o", [128, 3], dt.float16)
    wlo_d = dram_in("wlo", [128, 6], dt.float16)
    biases_d = dram_in("biases", [128, NB], dt.float32)
    m30_d = dram_in("m30", [3, 30], dt.float32)
    timeb_d = dram_in("timeb", [1, 64], dt.float32)

    ov_d = nc.dram_tensor("ov_out", [NPC, 4], dt.float32, kind="ExternalOutput")
    ol_d = nc.dram_tensor("ol_out", [GPAD, 6], dt.float32, kind="ExternalOutput")
    dbg_d = {}
    for nm, shape in debug_outs:
        dbg_d[nm] = nc.dram_tensor(nm, list(shape), dt.float32,
                                   kind="ExternalOutput")

    f16, f32, i32 = dt.float16, dt.float32, dt.int32
    f32r = dt.float32r
    AF = mybir.ActivationFunctionType
    OP = mybir.AluOpType
    PI = float(np.pi)

    from contextlib import ExitStack
    with tile.TileContext(nc) as tc, ExitStack() as stack:
        # ------- persistent SBUF state -------
        pers = stack.enter_context(tc.tile_pool(name="pers", bufs=1))
        nf_sb = pers.tile([128, NPC], f32)          # node features
        hn_sb = pers.tile([128, NPC], f16)          # layernormed (per layer)
        A_sb = pers.tile([128, NWIN * 128], f16)    # A window-major
        agg_fm = pers.tile([128, NPC], f16)
        rdeg_sb = pers.tile([128, NWIN], f32)
        ident32 = pers.tile([128, 128], f32)
        ident16 = pers.tile([128, 128], f16)
        iota128 = pers.tile([128, 128], f16)
        iota512 = pers.tile([128, 512], f16)
        biases_sb = pers.tile([128, NB], f32)
        negpi = pers.tile([128, 1], f32)
        pihalf = pers.tile([128, 1], f32)
        m30_sb = pers.tile([3, 30], f32)
        ones_row = pers.tile([1, 128], f32)
        ones_col = pers.tile([128, 1], f32)
        ones_row16 = pers.tile([1, 128], f16)
        ones_col16 = pers.tile([128, 1], f16)
        ovp_sb = pers.tile([3, NPC], f32)
        epsc = pers.tile([128, 1], f32)

        make_identity(nc, ident32[:])
        nc.vector.tensor_copy(out=ident16[:], in_=ident32[:])
        ii16 = pers.tile([128, 512], dt.int16)
        nc.gpsimd.iota(ii16[:, 0:128], pattern=[[1, 128]], base=0,
                       channel_multiplier=0)
        nc.vector.tensor_copy(out=iota128[:], in_=ii16[:, 0:128])
        nc.gpsimd.iota(ii16[:], pattern=[[1, 512]], base=0,
                       channel_multiplier=0)
        nc.vector.tensor_copy(out=iota512[:], in_=ii16[:])
        nc.sync.dma_start(out=biases_sb[:], in_=biases_d.ap())
        nc.gpsimd.memset(negpi[:], -PI)
        nc.gpsimd.memset(pihalf[:], PI / 2.0)
        nc.gpsimd.memset(epsc[:], EPS)
        nc.gpsimd.memset(ones_row[:], 1.0)
        nc.gpsimd.memset(ones_col[:], 1.0)
        nc.gpsimd.memset(ones_row16[:], 1.0)
        nc.gpsimd.memset(ones_col16[:], 1.0)
        nc.sync.dma_start(out=m30_sb[:], in_=m30_d.ap())
        nc.sync.dma_start(out=rdeg_sb[:], in_=rdeg_d.ap())
        nc.vector.memset(A_sb[:], 0.0)
        nc.vector.memset(agg_fm[:], 0.0)
        nc.vector.memset(ovp_sb[:], 0.0)

        # ------- DRAM internal -------
        dramp = stack.enter_context(tc.tile_pool(name="dram", bufs=1, space="DRAM"))
        temb_dram = dramp.tile([1024, 128], f32)
        comb_dram = dramp.tile([NT, 80, 512], f16)
        Aloc_dram = dramp.tile([NPC, 128], f16)
        Bloc_dram = dramp.tile([NPC, 128], f16)
        Bfull_drams = [dramp.tile([NCORES * NPC, 128], f16,
                                  addr_space="Shared", name=f"Bfull{i}",
                                  tag=f"Bfull{i}") for i in range(L)]
        Gpart_dram = dramp.tile([GPAD + 256, 132], f32)
        Gsum_dram = dramp.tile([GPAD + 256, 132], f32, addr_space="Shared")
        ovmean_dram = dramp.tile([GPAD, 4], f32)

        def bias(nm):
            return biases_sb[:, IB[nm]:IB[nm] + 1]

        # =========== preamble: temb ===========
        with tc.tile_pool(name="pre", bufs=2) as pool, \
             tc.tile_pool(name="prep", bufs=2, space="PSUM") as psp:
            tb = pool.tile([1, 64], f32)
            tr = pool.tile([1, 1024], f32)
            nc.sync.dma_start(out=tb[:], in_=timeb_d.ap())
            nc.sync.dma_start(out=tr[:], in_=trow_d.ap())
            projp = psp.tile([64, 1024], f32)
            nc.tensor.matmul(out=projp[:, 0:512], lhsT=tb[:], rhs=tr[:, 0:512],
                             start=True, stop=True)
            nc.tensor.matmul(out=projp[:, 512:1024], lhsT=tb[:],
                             rhs=tr[:, 512:1024], start=True, stop=True)
            proj_sb = pool.tile([64, 1024], f32)
            nc.vector.tensor_copy(out=proj_sb[:], in_=projp[:])
            for g8 in range(8):
                ptp = psp.tile([128, 64], f32, tag="ptp")
                nc.tensor.matmul(out=ptp[:], lhsT=proj_sb[:, g8 * 128:(g8 + 1) * 128],
                                 rhs=ident32[0:64, 0:64], is_transpose=True,
                                 start=True, stop=True)
                tg = pool.tile([128, 128], f32, tag="tg")
                nc.scalar.activation(out=tg[:, 0:64], in_=ptp[:], func=AF.Sin,
                                     scale=2.0 * PI)
                # cos(x) = sin(x + pi/2); args are tiny (|x|<~1) so no wrap
                cosb = pool.tile([128, 1], f32, tag="cosb")
                nc.gpsimd.memset(cosb[:], PI / 2.0)
                nc.scalar.activation(out=tg[:, 64:128], in_=ptp[:], func=AF.Sin,
                                     scale=2.0 * PI, bias=cosb[:])
                nc.sync.dma_start(out=temb_dram[g8 * 128:(g8 + 1) * 128, :],
                                  in_=tg[:])

        # =========== preamble: emb + t_per_atom + atom MLP -> nf ===========
        with tc.tile_pool(name="atom", bufs=3) as pool, \
             tc.tile_pool(name="atomp", bufs=3, space="PSUM") as psp:
            embsb = pool.tile([128, NPC], f16, bufs=1)
            aw0 = pool.tile([128, 128], f16, bufs=1)
            aw1 = pool.tile([128, 128], f16, bufs=1)
            nc.sync.dma_start(out=embsb[:], in_=embfm_d.ap())
            nc.sync.dma_start(out=aw0[:], in_=atomw_d.ap()[0])
            nc.sync.dma_start(out=aw1[:], in_=atomw_d.ap()[1])
            for k in range(49):
                tix = pool.tile([128, 1], i32, tag="tix")
                nc.sync.dma_start(out=tix[:], in_=tidx_d.ap()[k])
                te = pool.tile([128, 128], f32, tag="te")
                nc.gpsimd.indirect_dma_start(
                    out=te[:], out_offset=None, in_=temb_dram[:],
                    in_offset=bass.IndirectOffsetOnAxis(ap=tix[:, 0:1], axis=0))
                tep = psp.tile([128, 128], f32, tag="tep")
                nc.tensor.matmul(out=tep[:], lhsT=te[:], rhs=ident32[:],
                                 is_transpose=True, start=True, stop=True)
                te16 = pool.tile([128, 128], f16, tag="te16")
                nc.vector.tensor_copy(out=te16[:], in_=tep[:])
                nfp = psp.tile([128, 128], f32, tag="nfp")
                nc.tensor.matmul(out=nfp[:], lhsT=aw0[:],
                                 rhs=embsb[:, k * 128:(k + 1) * 128],
                                 start=True, stop=False)
                nc.tensor.matmul(out=nfp[:], lhsT=aw1[:], rhs=te16[:],
                                 start=False, stop=True)
                nc.scalar.activation(out=nf_sb[:, k * 128:(k + 1) * 128],
                                     in_=nfp[:], func=AF.Identity,
                                     bias=bias("atomb"))

        # =========== preamble: static edge features -> comb_dram ===========
        with tc.tile_pool(name="stat", bufs=4) as pool, \
             tc.tile_pool(name="statp", bufs=4, space="PSUM") as psp:
            for t in range(NT):
                pd0 = pool.tile([3, 512], f32, tag="pd0")
                pd1 = pool.tile([3, 512], f32, tag="pd1")
                nc.sync.dma_start(out=pd0[:], in_=spos_d.ap()[t, 0])
                nc.sync.dma_start(out=pd1[:], in_=spos_d.ap()[t, 1])
                ldv0 = pool.tile([128, 512], f32, tag="ldv0")
                ldv1 = pool.tile([128, 512], f32, tag="ldv1")
                nc.sync.dma_start(out=ldv0[64:76, :], in_=sldv_d.ap()[t, 0])
                nc.sync.dma_start(out=ldv1[64:76, :], in_=sldv_d.ap()[t, 1])
                comb = pool.tile([80, 512], f16, tag="comb")
                nc.vector.memset(comb[:], 0.0)
                pdf = pool.tile([3, 512], f32, tag="pdf")
                nc.vector.tensor_tensor(out=pdf[:], in0=pd0[:], in1=pd1[:],
                                        op=OP.subtract)
                nc.vector.tensor_tensor(out=comb[64:76, :], in0=ldv0[64:76, :],
                                        in1=ldv1[64:76, :], op=OP.subtract)
                a30 = psp.tile([64, 512], f32, tag="a30")
                nc.tensor.matmul(out=a30[0:30, :], lhsT=m30_sb[:], rhs=pdf[:],
                                 start=True, stop=True)
                nc.tensor.matmul(out=a30[32:62, :], lhsT=m30_sb[:], rhs=pdf[:],
                                 start=True, stop=True, tile_position=(0, 32))
                # range-reduce: m = a - 2*pi*round(a/(2*pi)) via the
                # round-to-nearest magic constant (1.5 * 2**23)
                MAGIC = 12582912.0
                INV2PI = 1.0 / (2.0 * PI)
                sarg = pool.tile([64, 512], f32, tag="sarg")
                karg = pool.tile([64, 512], f32, tag="karg")
                nc.vector.tensor_scalar(out=karg[0:30, :], in0=a30[0:30, :],
                                        scalar1=INV2PI, scalar2=MAGIC,
                                        op0=OP.mult, op1=OP.add)
                nc.vector.tensor_scalar(out=karg[32:62, :], in0=a30[32:62, :],
                                        scalar1=INV2PI, scalar2=MAGIC + 0.25,
                                        op0=OP.mult, op1=OP.add)
                nc.vector.tensor_scalar(out=karg[0:30, :], in0=karg[0:30, :],
                                        scalar1=MAGIC, scalar2=None,
                                        op0=OP.subtract)
                nc.vector.tensor_scalar(out=karg[32:62, :], in0=karg[32:62, :],
                                        scalar1=MAGIC, scalar2=None,
                                        op0=OP.subtract)
                nc.vector.scalar_tensor_tensor(
                    out=sarg[0:30, :], in0=karg[0:30, :], scalar=-2.0 * PI,
                    in1=a30[0:30, :], op0=OP.mult, op1=OP.add)
                nc.vector.scalar_tensor_tensor(
                    out=sarg[32:62, :], in0=karg[32:62, :], scalar=-2.0 * PI,
                    in1=a30[32:62, :], op0=OP.mult, op1=OP.add)
                nc.scalar.activation(out=comb[0:30, :], in_=sarg[0:30, :],
                                     func=AF.Sin)
                nc.scalar.activation(out=comb[32:62, :], in_=sarg[32:62, :],
                                     func=AF.Sin, bias=pihalf[32:62, :])
                nc.sync.dma_start(out=comb_dram[t], in_=comb[:])

        # =========== layer helper pieces ===========
        def layernorm(gname, bname, pool, lnps):
            """nf_sb (f32) -> hn_sb (f16)"""
            psp = lnps
            for (o, w) in NT_NODE:
                xt = nf_sb[:, o:o + w]
                nf16 = pool.tile([128, 512], f16, tag="lnx16")
                nc.vector.tensor_copy(out=nf16[:, 0:w], in_=xt)
                sq = pool.tile([128, 512], f16, tag="lnsq")
                nc.vector.tensor_tensor(out=sq[:, 0:w], in0=nf16[:, 0:w],
                                        in1=nf16[:, 0:w], op=OP.mult)
                s0 = psp.tile([1, 512], f32, tag="lns0")
                s1 = psp.tile([1, 512], f32, tag="lns1")
                nc.tensor.matmul(out=s0[:, 0:w], lhsT=ones_col16[:],
                                 rhs=nf16[:, 0:w], start=True, stop=True)
                nc.tensor.matmul(out=s1[:, 0:w], lhsT=ones_col16[:],
                                 rhs=sq[:, 0:w], start=True, stop=True)
                mean = pool.tile([1, 512], f32, tag="lnmean")
                nc.vector.tensor_scalar(out=mean[:, 0:w], in0=s0[:, 0:w],
                                        scalar1=1.0 / 128, scalar2=None,
                                        op0=OP.mult)
                ms = pool.tile([1, 512], f32, tag="lnms")
                nc.vector.tensor_tensor(out=ms[:, 0:w], in0=mean[:, 0:w],
                                        in1=mean[:, 0:w], op=OP.mult)
                var = pool.tile([1, 512], f32, tag="lnvar")
                nc.vector.scalar_tensor_tensor(
                    out=var[:, 0:w], in0=s1[:, 0:w], scalar=1.0 / 128,
                    in1=ms[:, 0:w], op0=OP.mult, op1=OP.subtract)
                sd = pool.tile([1, 512], f32, tag="lnsd")
                nc.scalar.activation(out=sd[:, 0:w], in_=var[:, 0:w],
                                     func=AF.Sqrt, bias=epsc[0:1, :])
                rstd = pool.tile([1, 512], f32, tag="lnrstd")
                nc.vector.reciprocal(out=rstd[:, 0:w], in_=sd[:, 0:w])
                srow = pool.tile([1, 512], f16, tag="lnsrow")
                nc.vector.scalar_tensor_tensor(
                    out=srow[:, 0:w], in0=mean[:, 0:w], scalar=-1.0,
                    in1=rstd[:, 0:w], op0=OP.mult, op1=OP.mult)
                rstd16 = pool.tile([1, 512], f16, tag="lnrstd16")
                nc.vector.tensor_copy(out=rstd16[:, 0:w], in_=rstd[:, 0:w])
                rb = psp.tile([128, 512], f32, tag="lnrb")
                sb = psp.tile([128, 512], f32, tag="lnsb")
                nc.tensor.matmul(out=rb[:, 0:w], lhsT=ones_row16[:],
                                 rhs=rstd16[:, 0:w], start=True, stop=True)
                nc.tensor.matmul(out=sb[:, 0:w], lhsT=ones_row16[:],
                                 rhs=srow[:, 0:w], start=True, stop=True)
                t1 = pool.tile([128, 512], f32, tag="lnt1")
                nc.vector.tensor_tensor(out=t1[:, 0:w], in0=xt, in1=rb[:, 0:w],
                                        op=OP.mult)
                t2 = pool.tile([128, 512], f32, tag="lnt2")
                nc.vector.tensor_tensor(out=t2[:, 0:w], in0=t1[:, 0:w],
                                        in1=sb[:, 0:w], op=OP.add)
                nc.scalar.activation(out=hn_sb[:, o:o + w], in_=t2[:, 0:w],
                                     func=AF.Identity, bias=bias(bname),
                                     scale=bias(gname))

        # =========== main layers ===========
        for li in range(layers):
            # ---- node stage: LN, A/B production, allgather ----
            with tc.tile_pool(name=f"ln{li}", bufs=3) as pool, \
                 tc.tile_pool(name=f"lnq{li}", bufs=1, space="PSUM") as lnps, \
                 tc.tile_pool(name=f"lnp{li}", bufs=2, space="PSUM") as psp:
                layernorm(f"lng_{li}", f"lnb_{li}", pool, lnps)
                w1ab = pool.tile([128, 256], f16, bufs=1, tag="w1ab")
                nc.sync.dma_start(out=w1ab[:], in_=w1ab_d.ap()[li])
                for w in range(NWIN):
                    a = w * WN
                    b = min(a + WN, NPC)
                    nn = b - a
                    abp = psp.tile([128, 256], f32, tag="abp")
                    nc.tensor.matmul(out=abp[0:nn, :], lhsT=hn_sb[:, a:b],
                                     rhs=w1ab[:], start=True, stop=True)
                    ab16 = pool.tile([128, 256], f16, tag="ab16")
                    nc.vector.tensor_copy(out=ab16[0:nn, :], in_=abp[0:nn, :])
                    nc.vector.tensor_copy(out=A_sb[0:nn, w * 128:(w + 1) * 128],
                                          in_=ab16[0:nn, 0:128])
                    nc.sync.dma_start(out=Bloc_dram[a:b, :],
                                      in_=ab16[0:nn, 128:256])
                nc.gpsimd.collective_compute(
                    "AllGather", OP.bypass,
                    replica_groups=[list(range(NCORES))],
                    ins=[Bloc_dram.opt()], outs=[Bfull_drams[li].opt()])

            # ---- edge stage ----
            with tc.tile_pool(name=f"ed{li}", bufs=4) as pool, \
                 tc.tile_pool(name=f"edp{li}", bufs=2, space="PSUM") as psp, \
                 tc.tile_pool(name=f"edpa{li}", bufs=1, space="PSUM") as pspa:
                wcomb = pool.tile([80, 128], f16, bufs=1, tag="wcomb")
                w2t = pool.tile([128, 128], f16, bufs=1, tag="w2t")
                nc.sync.dma_start(out=wcomb[:], in_=wcomb_d.ap()[li])
                nc.sync.dma_start(out=w2t[:], in_=w2t_d.ap()[li])
                for w in range(NWIN):
                    aggp = pspa.tile([128, 128], f32, tag="aggp")
                    for ti in range(TPC):
                        t = w * TPC + ti
                        meta_t = pool.tile([128, 8], f32, tag="meta")
                        nc.sync.dma_start(out=meta_t[:], in_=meta_d.ap()[t])
                        idx_t = pool.tile([128, 4], i32, tag="idx")
                        nc.sync.dma_start(out=idx_t[:], in_=bidx_d.ap()[t])
                        comb = pool.tile([80, 512], f16, tag="combl")
                        nc.sync.dma_start(out=comb[:], in_=comb_dram[t])
                        bg = pool.tile([128, 512], f16, tag="bgath")
                        for c in range(4):
                            nc.gpsimd.indirect_dma_start(
                                out=bg[:, c * 128:(c + 1) * 128],
                                out_offset=None, in_=Bfull_drams[li][:],
                                in_offset=bass.IndirectOffsetOnAxis(
                                    ap=idx_t[:, c:c + 1], axis=0))
                        bg32 = pool.tile([128, 512], f32, tag="bg32")
                        nc.vector.tensor_copy(out=bg32[:], in_=bg[:])
                        # A-expand interval onehot [node_window, edge]
                        oh = pool.tile([128, 512], f16, tag="ohfm")
                        ohx = pool.tile([128, 512], f16, tag="ohx")
                        nc.vector.tensor_scalar(
                            out=oh[:], in0=iota512[:], scalar1=meta_t[:, 4:5],
                            scalar2=None, op0=OP.is_ge)
                        nc.vector.tensor_scalar(
                            out=ohx[:], in0=iota512[:], scalar1=meta_t[:, 5:6],
                            scalar2=None, op0=OP.is_lt)
                        nc.vector.tensor_tensor(out=oh[:], in0=oh[:],
                                                in1=ohx[:], op=OP.mult)
                        eh = psp.tile([128, 512], f32, tag="eh")
                        nc.tensor.matmul(out=eh[:], lhsT=wcomb[:], rhs=comb[:],
                                         start=True, stop=False)
                        nc.tensor.matmul(
                            out=eh[:], lhsT=A_sb[:, w * 128:(w + 1) * 128],
                            rhs=oh[:], start=False, stop=False)
                        for c in range(4):
                            nc.tensor.matmul(
                                out=eh[:, c * 128:(c + 1) * 128].bitcast(f32),
                                lhsT=bg32[:, c * 128:(c + 1) * 128],
                                rhs=ident32[:], is_transpose=True,
                                start=False, stop=(c == 3),
                                skip_group_check=True)
                        e1 = pool.tile([128, 512], f16, tag="e1")
                        nc.scalar.activation(out=e1[:], in_=eh[:], func=AF.Silu,
                                             bias=bias(f"b1_{li}"))
                        e2p = psp.tile([128, 512], f32, tag="e2p")
                        nc.tensor.matmul(out=e2p[:], lhsT=w2t[:], rhs=e1[:],
                                         start=True, stop=True)
                        s2 = pool.tile([128, 512], f16, tag="s2")
                        nc.scalar.activation(out=s2[:], in_=e2p[:], func=AF.Silu,
                                             bias=bias(f"b2_{li}"))
                        s2tp = psp.tile([128, 512], f16, tag="s2tp")
                        for c in range(4):
                            nc.tensor.matmul(
                                out=s2tp[:, c * 128:(c + 1) * 128],
                                lhsT=s2[:, c * 128:(c + 1) * 128],
                                rhs=ident16[:], is_transpose=True,
                                start=True, stop=True)
                        s2t = pool.tile([128, 512], f16, tag="s2t")
                        nc.vector.tensor_copy(out=s2t[:], in_=s2tp[:])
                        ohe = pool.tile([128, 512], f16, tag="ohe")
                        for c in range(4):
                            nc.vector.tensor_scalar(
                                out=ohe[:, c * 128:(c + 1) * 128],
                                in0=iota128[:], scalar1=meta_t[:, c:c + 1],
                                scalar2=None, op0=OP.is_equal)
                        for c in range(4):
                            nc.tensor.matmul(
                                out=aggp[:],
                                lhsT=ohe[:, c * 128:(c + 1) * 128],
                                rhs=s2t[:, c * 128:(c + 1) * 128],
                                start=(ti == 0 and c == 0),
                                stop=(ti == TPC - 1 and c == 3))
                    # window done: scale & transpose into agg_fm
                    a = w * WN
                    b = min(a + WN, NPC)
                    nn = b - a
                    asc = pool.tile([128, 128], f16, tag="asc")
                    nc.vector.tensor_scalar(out=asc[:], in0=aggp[:],
                                            scalar1=rdeg_sb[:, w:w + 1],
                                            scalar2=None, op0=OP.mult)
                    atp = pspa.tile([128, 128], f16, tag="atp")
                    nc.tensor.matmul(out=atp[:], lhsT=asc[:], rhs=ident16[:],
                                     is_transpose=True, start=True, stop=True)
                    nc.vector.tensor_copy(out=agg_fm[:, a:b], in_=atp[:, 0:nn])

            # ---- node MLP + residual ----
            with tc.tile_pool(name=f"nm{li}", bufs=3) as pool, \
                 tc.tile_pool(name=f"nmp{li}", bufs=3, space="PSUM") as psp:
                wn1a = pool.tile([128, 128], f16, bufs=1, tag="wn1a")
                wn1b = pool.tile([128, 128], f16, bufs=1, tag="wn1b")
                wn2 = pool.tile([128, 128], f16, bufs=1, tag="wn2")
                nc.sync.dma_start(out=wn1a[:], in_=wn1_d.ap()[li, 0])
                nc.sync.dma_start(out=wn1b[:], in_=wn1_d.ap()[li, 1])
                nc.sync.dma_start(out=wn2[:], in_=wn2_d.ap()[li])
                for (o, w) in NT_NODE:
                    o1p = psp.tile([128, 512], f32, tag="o1p")
                    nc.tensor.matmul(out=o1p[:, 0:w], lhsT=wn1a[:],
                                     rhs=hn_sb[:, o:o + w], start=True,
                                     stop=False)
                    nc.tensor.matmul(out=o1p[:, 0:w], lhsT=wn1b[:],
                                     rhs=agg_fm[:, o:o + w], start=False,
                                     stop=True)
                    o1 = pool.tile([128, 512], f16, tag="o1")
                    nc.scalar.activation(out=o1[:, 0:w], in_=o1p[:, 0:w],
                                         func=AF.Silu, bias=bias(f"bn1_{li}"))
                    o2p = psp.tile([128, 512], f32, tag="o2p")
                    nc.tensor.matmul(out=o2p[:, 0:w], lhsT=wn2[:],
                                     rhs=o1[:, 0:w], start=True, stop=True)
                    o2 = pool.tile([128, 512], f32, tag="o2")
                    nc.scalar.activation(out=o2[:, 0:w], in_=o2p[:, 0:w],
                                         func=AF.Silu, bias=bias(f"bn2_{li}"))
                    nc.vector.tensor_tensor(out=nf_sb[:, o:o + w],
                                            in0=nf_sb[:, o:o + w],
                                            in1=o2[:, 0:w], op=OP.add)

        if "nf" in dbg_d:
            with tc.tile_pool(name="dbgnf", bufs=2) as pool:
                for (o, w) in NT_NODE:
                    nc.sync.dma_start(out=dbg_d["nf"].ap()[:, o:o + w],
                                      in_=nf_sb[:, o:o + w])

        # =========== final: LN + heads + graph phase ===========
        with tc.tile_pool(name="fin", bufs=3) as pool, \
             tc.tile_pool(name="finq", bufs=1, space="PSUM") as lnps, \
             tc.tile_pool(name="finp", bufs=2, space="PSUM") as psp:
            layernorm("flng", "flnb", pool, lnps)
            w1o = pool.tile([128, 128], f16, bufs=1, tag="w1o")
            w2o = pool.tile([128, 3], f16, bufs=1, tag="w2o")
            nc.sync.dma_start(out=w1o[:], in_=w1o_d.ap())
            nc.sync.dma_start(out=w2o[:], in_=w2o_d.ap())
            for (o, w) in NT_NODE:
                h1p = psp.tile([128, 512], f32, tag="h1p")
                nc.tensor.matmul(out=h1p[:, 0:w], lhsT=w1o[:],
                                 rhs=hn_sb[:, o:o + w], start=True, stop=True)
                h1 = pool.tile([128, 512], f16, tag="h1")
                nc.scalar.activation(out=h1[:, 0:w], in_=h1p[:, 0:w],
                                     func=AF.Silu, bias=bias("b1o"))
                ovpp = psp.tile([3, 512], f32, tag="ovpp")
                nc.tensor.matmul(out=ovpp[:, 0:w], lhsT=w2o[:],
                                 rhs=h1[:, 0:w], start=True, stop=True)
                nc.vector.tensor_copy(out=ovp_sb[:, o:o + w],
                                      in_=ovpp[:, 0:w])

        # graph partial sums
        with tc.tile_pool(name="gph", bufs=3) as pool, \
             tc.tile_pool(name="gphp", bufs=2, space="PSUM") as psp, \
             tc.tile_pool(name="gphq", bufs=1, space="PSUM") as pspq:
            gps = [pspq.tile([128, 131], f32, tag=f"gps{w}", name=f"gps{w}") for w in range(3)]
            for k in range(49):
                ovtp = psp.tile([128, 3], f32, tag="ovtp")
                nc.tensor.matmul(out=ovtp[:], lhsT=ovp_sb[:, k * 128:(k + 1) * 128],
                                 rhs=ident32[0:3, 0:3], is_transpose=True,
                                 start=True, stop=True)
                htp = psp.tile([128, 128], f16, tag="htp")
                nc.tensor.matmul(out=htp[:], lhsT=hn_sb[:, k * 128:(k + 1) * 128],
                                 rhs=ident16[:], is_transpose=True,
                                 start=True, stop=True)
                rhs = pool.tile([128, 131], f16, tag="grhs")
                nc.vector.tensor_copy(out=rhs[:, 0:3], in_=ovtp[:])
                nc.vector.tensor_copy(out=rhs[:, 3:131], in_=htp[:])
                gw = pool.tile([128, 3], f32, tag="gw")
                nc.sync.dma_start(out=gw[:], in_=gidxw_d.ap()[k])
                for w in range(3):
                    ohg = pool.tile([128, 128], f16, tag="ohg")
                    nc.vector.tensor_scalar(out=ohg[:], in0=iota128[:],
                                            scalar1=gw[:, w:w + 1],
                                            scalar2=None, op0=OP.is_equal)
                    nc.tensor.matmul(out=gps[w][:], lhsT=ohg[:], rhs=rhs[:],
                                     start=(k == 0), stop=(k == 48))
            # zero Gpart then write windows at dynamic offset
            zt = pool.tile([128, 132], f32, tag="zt", bufs=1)
            nc.gpsimd.memset(zt[:], 0.0)
            for k in range((GPAD + 256) // 128):
                nc.sync.dma_start(out=Gpart_dram[k * 128:(k + 1) * 128, :],
                                  in_=zt[:])
            gb = pool.tile([1, 8], i32, bufs=1, tag="gb")
            nc.sync.dma_start(out=gb[:], in_=gbase_d.ap())
            base_reg = nc.sync.alloc_register("gwbase")
            nc.sync.reg_load(base_reg, gb[0:1, 0:1])
            base_sv = nc.snap(base_reg)
            for w in range(3):
                gsb = pool.tile([128, 131], f32, tag="gsb")
                nc.vector.tensor_copy(out=gsb[:], in_=gps[w][:])
                nc.sync.dma_start(
                    out=Gpart_dram[bass.ds(base_sv + w * 128, 128), 0:131],
                    in_=gsb[:])
            nc.gpsimd.collective_compute(
                "AllReduce", OP.add, replica_groups=[list(range(NCORES))],
                ins=[Gpart_dram.opt()], outs=[Gsum_dram.opt()])

        # graph means, ol, ov
        with tc.tile_pool(name="gm", bufs=3) as pool, \
             tc.tile_pool(name="gmp", bufs=2, space="PSUM") as psp:
            wlo = pool.tile([128, 6], f16, bufs=1, tag="wlo")
            nc.sync.dma_start(out=wlo[:], in_=wlo_d.ap())
            for gc in range(GPAD // 128):
                gs = pool.tile([128, 131], f32, tag="gs")
                nc.sync.dma_start(out=gs[:], in_=Gsum_dram[gc * 128:(gc + 1) * 128, 0:131])
                rc = pool.tile([128, 1], f32, tag="rc")
                nc.sync.dma_start(out=rc[:], in_=recipg_d.ap()[gc * 128:(gc + 1) * 128])
                gm = pool.tile([128, 131], f32, tag="gmt")
                nc.vector.tensor_scalar(out=gm[:], in0=gs[:], scalar1=rc[:, 0:1],
                                        scalar2=None, op0=OP.mult)
                nc.sync.dma_start(out=ovmean_dram[gc * 128:(gc + 1) * 128, 0:3],
                                  in_=gm[:, 0:3])
                gmf = pool.tile([128, 128], f16, tag="gmf")
                nc.vector.tensor_copy(out=gmf[:], in_=gm[:, 3:131])
                gft = psp.tile([128, 128], f16, tag="gft")
                nc.tensor.matmul(out=gft[:], lhsT=gmf[:], rhs=ident16[:],
                                 is_transpose=True, start=True, stop=True)
                gfts = pool.tile([128, 128], f16, tag="gfts")
                nc.vector.tensor_copy(out=gfts[:], in_=gft[:])
                olp = psp.tile([128, 6], f32, tag="olp")
                nc.tensor.matmul(out=olp[:], lhsT=gfts[:], rhs=wlo[:],
                                 start=True, stop=True)
                ols = pool.tile([128, 6], f32, tag="ols")
                nc.vector.tensor_copy(out=ols[:], in_=olp[:])
                nc.sync.dma_start(out=ol_d.ap()[gc * 128:(gc + 1) * 128, :],
                                  in_=ols[:])
            for k in range(49):
                tix = pool.tile([128, 1], i32, tag="tix2")
                nc.sync.dma_start(out=tix[:], in_=tidx_d.ap()[k])
                om = pool.tile([128, 4], f32, tag="om")
                nc.gpsimd.indirect_dma_start(
                    out=om[:], out_offset=None, in_=ovmean_dram[:],
                    in_offset=bass.IndirectOffsetOnAxis(ap=tix[:, 0:1], axis=0))
                ovtp2 = psp.tile([128, 3], f32, tag="ovtp2")
                nc.tensor.matmul(out=ovtp2[:], lhsT=ovp_sb[:, k * 128:(k + 1) * 128],
                                 rhs=ident32[0:3, 0:3], is_transpose=True,
                                 start=True, stop=True)
                oo = pool.tile([128, 4], f32, tag="oo")
                nc.vector.tensor_tensor(out=oo[:, 0:3], in0=ovtp2[:],
                                        in1=om[:, 0:3], op=OP.subtract)
                nc.sync.dma_start(out=ov_d.ap()[k * 128:(k + 1) * 128, 0:3],
                                  in_=oo[:, 0:3])


    nc.compile()
    return nc


# ---------------------------------------------------------------------------
# runner
# ---------------------------------------------------------------------------

def _make_runner(nc, n_cores=NCORES):
    import jax
    import concourse.mybir as mybir
    from jax.sharding import Mesh, PartitionSpec, NamedSharding
    from jax.experimental.shard_map import shard_map
    from concourse.bass2jax import (_bass_exec_p, install_neuronx_cc_hook,
                                    partition_id_tensor)

    install_neuronx_cc_hook()
    partition_name = nc.partition_id_tensor.name if nc.partition_id_tensor else None
    in_names, out_names, out_avals, zero_outs = [], [], [], []
    for alloc in nc.m.functions[0].allocations:
        if not isinstance(alloc, mybir.MemoryLocationSet):
            continue
        name = alloc.memorylocations[0].name
        if alloc.kind == "ExternalInput":
            if name != partition_name:
                in_names.append(name)
        elif alloc.kind == "ExternalOutput":
            shape = tuple(alloc.tensor_shape)
            dtype = mybir.dt.np(alloc.dtype)
            out_names.append(name)
            out_avals.append(jax.core.ShapedArray(shape, dtype))
            zero_outs.append(np.zeros(shape, dtype))
    n_params = len(in_names)
    n_outs = len(out_avals)
    in_names_all = in_names + out_names
    if partition_name is not None:
        in_names_all = in_names_all + [partition_name]
    donate = tuple(range(n_params, n_params + n_outs))

    def _body(*args):
        operands = list(args)
        if partition_name is not None:
            operands.append(partition_id_tensor())
        outs = _bass_exec_p.bind(
            *operands, out_avals=tuple(out_avals), in_names=tuple(in_names_all),
            out_names=tuple(out_names),
            lowering_input_output_aliases=(),
            sim_require_finite=False, sim_require_nnan=False, nc=nc)
        return tuple(outs)

    devices = jax.devices()[:n_cores]
    mesh = Mesh(np.asarray(devices), ("core",))
    in_specs = (PartitionSpec("core"),) * (n_params + n_outs)
    out_specs = (PartitionSpec("core"),) * len(out_names)
    sharded = jax.jit(
        shard_map(_body, mesh=mesh, in_specs=in_specs, out_specs=out_specs,
                  check_rep=False),
        donate_argnums=donate, keep_unused=True)
    sh = NamedSharding(mesh, PartitionSpec("core"))

    def run(in_maps, n_time=1):
        import jax as _jax
        per_core = [[np.asarray(m[name]) for name in in_names] for m in in_maps]
        concat_in = [np.concatenate([per_core[c][i] for c in range(n_cores)],
                                    axis=0) for i in range(n_params)]
        concat_in = [_jax.device_put(a, sh) for a in concat_in]
        for a in concat_in:
            a.block_until_ready()
        times = []
        out_arrs = None
        for _ in range(n_time):
            concat_zeros = [
                _jax.device_put(np.zeros((n_cores * z.shape[0], *z.shape[1:]),
                                         z.dtype), sh) for z in zero_outs]
            for a in concat_zeros:
                a.block_until_ready()
            t0 = time.time()
            out_arrs = sharded(*concat_in, *concat_zeros)
            for o in out_arrs:
                o.block_until_ready()
            times.append(time.time() - t0)
        results = [
            {name: np.asarray(out_arrs[i]).reshape(n_cores, *out_avals[i].shape)[c]
             for i, name in enumerate(out_names)}
            for c in range(n_cores)]
        return results, times

    return run


def _in_maps(host):
    maps = []
    for c in range(NCORES):
        maps.append({
            "meta": host["meta"][c], "bidx": host["bidx"][c],
            "spos": host["stat_pos"][c], "sldv": host["stat_ldv"][c],
            "rdeg": host["recipdeg"][c], "gidxw": host["gidxw"][c],
            "gbase": host["gbase"][c][None, :], "tidx": host["tidx"][c],
            "embfm": host["embfm"][c], "recipg": host["recipg"],
            "trow": host["trow"], "wcomb": host["Wcomb"],
            "w1ab": host["W1ab"], "w2t": host["W2T"], "wn1": host["Wn1"],
            "wn2": host["Wn2"], "atomw": host["atomw"], "w1o": host["W1o"],
            "w2o": host["W2o"], "wlo": host["Wlo"], "biases": host["biases"],
            "m30": host["M30"], "timeb": host["timeB"],
        })
    return maps


_CACHE = {}


def kernel(**inputs):
    host = preprocess(inputs)
    key = (host["NT"], host["TPC"])
    if key not in _CACHE:
        nc = build_program(host)
        _CACHE[key] = _make_runner(nc)
    run = _CACHE[key]
    results, _ = run(_in_maps(host), n_time=1)
    ov = np.concatenate([results[c]["ov_out"][:NPCR, 0:3] for c in range(NCORES)],
                        axis=0).astype(np.float32)
    ol = results[0]["ol_out"][:G].astype(np.float32)
    return ov, ol
